# revision 1
# baseline (speedup 1.0000x reference)
"""DefocusBlur on 8 NeuronCores (Trainium2, Bass/Tile).

Depthwise 17x17 disk-blur of images [32,3,512,512] f32, reflect-101 pad.

Sharding: pure data parallel over batch — 4 images (12 planes) per core.

Per-core algorithm: the 2D conv is decomposed per kernel column j into a
1-D conv along H (as a PSUM-accumulated banded matmul, contraction over
128 padded input rows) with the W-shift j applied as a free-axis offset
into the W-padded input tile. The disk kernel is left-right symmetric
(kcol_j == kcol_{16-j}), so mirror pairs are pre-summed on the vector
engine and share one matmul each: all 4 (or 5, on 3 of 4 blocks) pair
sums are computed by ONE wide DVE op using overlapping-window 3D APs
(in0 stride +1 from col 0, in1 stride -1 from col 16), amortizing the
per-op overhead; the promote fraction 42/57 is the PE/DVE balance
point for the fused-add cost. Inputs are
reflect-padded by 8 on the host so no edge logic runs on device.
Matmuls run as float32r (full PE rate at N=512, ~1e-4 rel err).

Schedule details (from cost-model trace analysis): all single-column
matmuls issue first, pair-matmuls last (with the alternating 4/5 pairing
and triple-buffered s-tiles, DVE runs ahead so PE never waits); weights
load as per-group chunks on the scalar-engine HWDGE ring (parallel to
input loads on the sync ring) ordered by first use; output DMAs also
ride the scalar ring; a short dummy-matmul stream warms the PE clock
gate (HAM) during the initial DMA wait. All 12 padded planes are
processed as one flat 6336-row space with M=112 blocks tiled across
plane boundaries (banded weights are translation-invariant); the
16-row pad-seam outputs are computed but never stored.
"""
import dataclasses

import numpy as np

_RADIUS = 8
_B, _C, _H, _W = 32, 3, 512, 512
_NCORES = 8
_PLANES = (_B // _NCORES) * _C
_M = 112
_KIN = _M + 2 * _RADIUS
_NBLK = 5
_HP = _H + 2 * _RADIUS
_WP = _W + 2 * _RADIUS

NPAIR = 4  # pairs pre-summed on DVE; groups = 17 - NPAIR


def _disk_kernel():
    L = np.arange(-8, 9)
    X, Y = np.meshgrid(L, L)
    disk = ((X ** 2 + Y ** 2) <= _RADIUS ** 2).astype(np.float32)
    disk /= disk.sum()
    x = np.arange(3, dtype=np.float32) - 1
    g = np.exp(-(x ** 2) / (2.0 * 0.5 ** 2))
    g /= g.sum()
    k2 = np.outer(g, g).astype(np.float32)
    p = np.pad(disk, 1, mode="reflect")
    out = np.zeros_like(disk)
    for i in range(3):
        for j in range(3):
            out += k2[i, j] * p[i : i + 17, j : j + 17]
    return out


def _groups():
    """Returns list of (cols, kcol_index): cols = list of W-shifts sharing
    banded weight kcol_index."""
    gs = []
    for j in range(NPAIR):
        gs.append(([j, 16 - j], j))
    for j in range(NPAIR, 17 - NPAIR):
        gs.append(([j], j))
    return gs


def _banded_weights():
    k2d = _disk_kernel()
    ws = []
    for _, j in _groups():
        B = np.zeros((_KIN, _M), np.float32)
        for c in range(_M):
            B[c : c + 17, c] = k2d[:, j]
        ws.append(B)
    return np.ascontiguousarray(np.concatenate(ws, axis=1))


_NC_CACHE = []


def _build_program():
    import concourse.bacc as bacc
    import concourse.mybir as mybir
    import concourse.tile as tile

    f32 = mybir.dt.float32
    f32r = mybir.dt.float32r
    gs = _groups()
    ng = len(gs)

    nc = bacc.Bacc("TRN2", target_bir_lowering=False, debug=False)
    x_d = nc.dram_tensor("x", [_PLANES, _HP, _WP], f32r, kind="ExternalInput")
    w_d = nc.dram_tensor("w", [_KIN, ng * _M], f32r, kind="ExternalInput")
    o_d = nc.dram_tensor("o", [_PLANES, _H, _W], f32, kind="ExternalOutput")

    with tile.TileContext(nc) as tc:
        with (
            tc.tile_pool(name="wpool", bufs=1) as wpool,
            tc.tile_pool(name="inp", bufs=3) as inp,
            tc.tile_pool(name="spool", bufs=4) as spool,
            tc.tile_pool(name="outp", bufs=3) as outp,
            tc.tile_pool(name="ps", bufs=3, space="PSUM") as psp,
        ):
            wt = wpool.tile([_KIN, ng * _M], f32r)
            # HAM warm-up: keep PE busy during the initial DMA wait so the
            # first real matmuls run at full clock.
            warm = wpool.tile([128, 64], f32)
            nc.gpsimd.memset(warm[:], 0.0)
            wps = psp.tile([64, 64], f32, tag="warm")
            for wi in range(12):
                nc.tensor.matmul(
                    wps[:], warm[:, :64], warm[:, :64],
                    start=(wi == 0), stop=(wi == 11),
                )
            w_loaded = [False]
            GH = _PLANES * _HP              # 6336 global padded rows
            NSTART = GH - 2 * _RADIUS       # 6320 window starts
            xf = x_d.rearrange("p h w -> (p h) w")
            nblocks = (NSTART + _M - 1) // _M
            for b in range(nblocks):
                    g0 = b * _M
                    mb = min(_M, NSTART - g0)
                    kb = mb + 2 * _RADIUS
                    xt = inp.tile([_KIN, _WP], f32r, tag="xt")
                    nc.sync.dma_start(xt[:kb, :], xf[g0 : g0 + kb, :])
                    if not w_loaded[0]:
                        w_loaded[0] = True
                        worder = (
                            [4] + [0] + [5, 6] + [1] + [7, 8] + [2]
                            + [9, 10, 11, 12] + [3]
                        )
                        for wg in worder:
                            nc.scalar.dma_start(
                                wt[:, wg * _M : (wg + 1) * _M],
                                w_d[:, wg * _M : (wg + 1) * _M],
                            )
                    promote = (b % 4 != 0)
                    npr = 5 if promote else 4
                    st4 = spool.tile([_KIN, 5, _W], f32r, tag="st4")
                    full = xt[:kb, :]
                    pdim = list(full.ap[0])
                    in0 = dataclasses.replace(
                        full, ap=[pdim, [1, npr], [1, _W]]
                    )
                    in1 = dataclasses.replace(
                        full, offset=full.offset + 16,
                        ap=[pdim, [-1, npr], [1, _W]],
                    )
                    nc.vector.tensor_add(st4[:kb, :npr], in0, in1)
                    ps = psp.tile([_M, _W], f32, tag="ps")
                    singles = [gi for gi, (c, _) in enumerate(gs) if len(c) == 1]
                    pairs = [gi for gi, (c, _) in enumerate(gs) if len(c) == 2]
                    order = singles + pairs
                    mms = []
                    for gi in order:
                        cols, _ = gs[gi]
                        if promote and gi == 4:
                            continue
                        elif promote and gi == 12:
                            continue
                        elif len(cols) == 1:
                            mms.append((gi, xt[:kb, cols[0] : cols[0] + _W]))
                        else:
                            mms.append((gi, st4[:kb, gi, :]))
                    if promote:
                        mms.append((4, st4[:kb, 4, :]))
                    for mi, (gi, rhs) in enumerate(mms):
                        nc.tensor.matmul(
                            ps[:mb, :],
                            wt[:kb, gi * _M : gi * _M + mb],
                            rhs,
                            start=(mi == 0),
                            stop=(mi == len(mms) - 1),
                        )
                    ot = outp.tile([_M, _W], f32, tag="ot")
                    nc.scalar.copy(ot[:mb, :], ps[:mb, :])
                    # store only valid output runs (skip pad-seam rows)
                    for p in range(_PLANES):
                        lo = max(g0, p * _HP)
                        hi = min(g0 + mb, p * _HP + _H)
                        if lo < hi:
                            nc.scalar.dma_start(
                                o_d[p, lo - p * _HP : hi - p * _HP, :],
                                ot[lo - g0 : hi - g0, :],
                            )
    nc.compile()
    return nc


def _get_program():
    if not _NC_CACHE:
        _NC_CACHE.append(_build_program())
    return _NC_CACHE[0]


def kernel(images: np.ndarray) -> np.ndarray:
    from concourse.bass_utils import run_bass_kernel_spmd

    images = np.asarray(images, dtype=np.float32)
    padded = np.pad(
        images, ((0, 0), (0, 0), (_RADIUS, _RADIUS), (_RADIUS, _RADIUS)),
        mode="reflect",
    )
    shards = np.ascontiguousarray(padded.reshape(_NCORES, _PLANES, _HP, _WP))
    w = _banded_weights()
    nc = _get_program()
    in_maps = [{"x": shards[c], "w": w} for c in range(_NCORES)]
    res = run_bass_kernel_spmd(nc, in_maps, list(range(_NCORES)))
    out = np.stack([res.results[c]["o"] for c in range(_NCORES)], axis=0)
    return np.ascontiguousarray(out.reshape(_B, _C, _H, _W).astype(np.float32))

